# revision 6
# baseline (speedup 1.0000x reference)
"""Trainium2 Bass kernel for nn_Lorenz96DBF: 8-core data-parallel over batch.

Device (per core, SPMD): encoder GEMMs  tanh(X@W1+b1)@W2+b2  and decoder
GEMMs tanh(Z@V1+c1)@V2+c2 on the TensorEngine with fused bias+tanh PSUM
eviction. Host: per-2x2-block Kalman recursion (closed form), KL, reparam
sampling, loss reductions (cheap: ~50M flops vs ~54G in the GEMMs).
"""
import math
import sys

import numpy as np

sys.path.insert(0, "/opt/trn_rl_repo")

import concourse.bass as bass  # noqa: E402
import concourse.tile as tile  # noqa: E402
from concourse import bacc, mybir  # noqa: E402
from concourse.bass_utils import run_bass_kernel_spmd  # noqa: E402

F32 = mybir.dt.float32

B, T, OBS, LAT, HID = 64, 200, 256, 512, 1024
NB = LAT // 2
NCORES = 8
BL = B // NCORES          # batches per core
NTOK = BL * T             # tokens per core
LOG_Q = -2.0
MAX_G = 100.0
INIT_COV = 10.0
Q = math.exp(LOG_Q)

_CACHE = {}


def _build_mlp(name, K1, M1, K2, M2, ntok, act_mid=True):
    """Program: out = (tanh(W1p.T @ x + b1) if act_mid) chained into W2p.T @ . + b2.

    x: (K1, ntok) DRAM.  W1p: (K1, M1). W2p: (M1==K2, M2). out: (M2, ntok).
    All fp32. Returns (nc, names).
    """
    nc = bacc.Bacc(None, target_bir_lowering=False, debug=False)
    P = 128
    NT = 512  # n-tile
    n_tiles = [(i * NT, min(NT, ntok - i * NT)) for i in range((ntok + NT - 1) // NT)]
    k1t, m1t, k2t, m2t = K1 // P, M1 // P, K2 // P, M2 // P

    with tile.TileContext(nc) as tc:
        with tc.tile_pool(name="dram", bufs=1, space="DRAM") as dram, \
             tc.tile_pool(name="w", bufs=1) as wp, \
             tc.tile_pool(name="xin", bufs=1) as xp, \
             tc.tile_pool(name="mid", bufs=1) as hp, \
             tc.tile_pool(name="outp", bufs=3) as op, \
             tc.tile_pool(name="ps", bufs=4, space="PSUM") as psp:
            x_d = dram.tile([K1, ntok], F32, kind="ExternalInput")
            w1_d = dram.tile([K1, M1], F32, kind="ExternalInput")
            b1_d = dram.tile([1, M1], F32, kind="ExternalInput")
            w2_d = dram.tile([K2, M2], F32, kind="ExternalInput")
            b2_d = dram.tile([1, M2], F32, kind="ExternalInput")
            o_d = dram.tile([M2, ntok], F32, kind="ExternalOutput")

            # load inputs
            x_sb = xp.tile([P, k1t, ntok], F32)
            for k in range(k1t):
                nc.sync.dma_start(out=x_sb[:, k], in_=x_d[k * P:(k + 1) * P, :])
            w1_sb = wp.tile([P, k1t, M1], F32)
            for k in range(k1t):
                nc.sync.dma_start(out=w1_sb[:, k], in_=w1_d[k * P:(k + 1) * P, :])
            w2_sb = wp.tile([P, k2t, M2], F32)
            for k in range(k2t):
                nc.sync.dma_start(out=w2_sb[:, k], in_=w2_d[k * P:(k + 1) * P, :])
            # biases: (128, m1t) layout so column m gives per-partition scalar
            b1_sb = wp.tile([P, m1t], F32)
            nc.sync.dma_start(
                out=b1_sb[:],
                in_=bass.AP(tensor=b1_d.tensor, offset=b1_d.offset,
                            ap=[[1, P], [P, m1t]]))
            b2_sb = wp.tile([P, m2t], F32)
            nc.sync.dma_start(
                out=b2_sb[:],
                in_=bass.AP(tensor=b2_d.tensor, offset=b2_d.offset,
                            ap=[[1, P], [P, m2t]]))

            h_sb = hp.tile([P, m1t, ntok], F32)

            # ---- GEMM 1: h = tanh(W1.T @ x + b1) ----
            for m in range(m1t):
                for (n0, nn) in n_tiles:
                    ps = psp.tile([P, NT], F32, tag="ps")
                    for k in range(k1t):
                        nc.tensor.matmul(
                            ps[:, :nn],
                            w1_sb[:, k, m * P:(m + 1) * P],
                            x_sb[:, k, n0:n0 + nn],
                            start=(k == 0), stop=(k == k1t - 1))
                    nc.scalar.activation(
                        h_sb[:, m, n0:n0 + nn], ps[:, :nn],
                        mybir.ActivationFunctionType.Tanh,
                        bias=b1_sb[:, m:m + 1], scale=1.0)

            # ---- GEMM 2: out = W2.T @ h + b2 ----
            for m in range(m2t):
                for (n0, nn) in n_tiles:
                    ps = psp.tile([P, NT], F32, tag="ps2")
                    for k in range(k2t):
                        nc.tensor.matmul(
                            ps[:, :nn],
                            w2_sb[:, k, m * P:(m + 1) * P],
                            h_sb[:, k, n0:n0 + nn],
                            start=(k == 0), stop=(k == k2t - 1))
                    o_sb = op.tile([P, NT], F32, tag="o")
                    nc.vector.tensor_scalar_add(o_sb[:, :nn], ps[:, :nn],
                                                b2_sb[:, m:m + 1])
                    nc.sync.dma_start(out=o_d[m * P:(m + 1) * P, n0:n0 + nn],
                                      in_=o_sb[:, :nn])

            names = dict(x=x_d.tensor.name, w1=w1_d.tensor.name,
                         b1=b1_d.tensor.name, w2=w2_d.tensor.name,
                         b2=b2_d.tensor.name, out=o_d.tensor.name)
    nc.compile()
    return nc, names


def _get_programs():
    if "enc" not in _CACHE:
        _CACHE["enc"] = _build_mlp("enc", OBS, HID, HID, 2 * LAT, NTOK)
        _CACHE["dec"] = _build_mlp("dec", LAT, HID, HID, OBS, NTOK)
    return _CACHE["enc"], _CACHE["dec"]


def _run(prog, per_core_feeds):
    nc, names = prog
    in_maps = []
    for feeds in per_core_feeds:
        in_maps.append({names[k]: np.ascontiguousarray(v, np.float32)
                        for k, v in feeds.items()})
    res = run_bass_kernel_spmd(nc, in_maps, list(range(NCORES)))
    return [r[names["out"]] for r in res.results]


def kernel(obs_seq, target_seq, lambdas, log_R, eps, W1, b1, W2, b2, V1, c1, V2, c2):
    obs_seq = np.asarray(obs_seq, np.float32)
    target_seq = np.asarray(target_seq, np.float32)
    lambdas = np.asarray(lambdas, np.float64)
    log_R = np.asarray(log_R, np.float64)
    eps = np.asarray(eps, np.float64)
    W1 = np.asarray(W1, np.float32)
    W2 = np.asarray(W2, np.float32)
    V1 = np.asarray(V1, np.float32)
    V2 = np.asarray(V2, np.float32)
    b1v = np.asarray(b1, np.float32).reshape(1, HID)
    b2v = np.asarray(b2, np.float32).reshape(1, 2 * LAT)
    c1v = np.asarray(c1, np.float32).reshape(1, HID)
    c2v = np.asarray(c2, np.float32).reshape(1, OBS)

    enc_prog, dec_prog = _get_programs()

    # ---- device: encoder ----
    feeds = []
    for cidx in range(NCORES):
        xs = obs_seq[cidx * BL:(cidx + 1) * BL].reshape(NTOK, OBS).T
        feeds.append(dict(x=xs, w1=W1, b1=b1v, w2=W2, b2=b2v))
    enc_outs = _run(enc_prog, feeds)  # each (2*LAT, NTOK)

    # ---- host: Kalman + KL + sampling (fp64) ----
    lp = lambdas.reshape(NB, 2)
    r = 1.0 / (1.0 + np.exp(-lp[:, 0]))
    th = lp[:, 1]
    cos, sin = np.cos(th), np.sin(th)
    rc, rs = r * cos, r * sin
    r2 = r * r
    p11, p22, p12 = rc * rc, rs * rs, rc * rs
    dq = p11 - p22

    kl_sum = 0.0
    z_all = []
    for cidx in range(NCORES):
        enc = enc_outs[cidx].astype(np.float64)  # (1024, NTOK)
        enc = enc.reshape(2 * LAT, BL, T)
        f1 = enc[0:LAT:2]            # (NB, BL, T)
        f2 = enc[1:LAT:2]
        gr1 = enc[LAT:2 * LAT:2]
        gr2 = enc[LAT + 1:2 * LAT:2]
        g1 = MAX_G * np.tanh(gr1 * gr1 / MAX_G)
        g2 = MAX_G * np.tanh(gr2 * gr2 / MAX_G)
        gf1 = g1 * f1
        gf2 = g2 * f2

        R2 = r2[:, None]
        RC = rc[:, None]
        RS = rs[:, None]
        P12c = p12[:, None]
        DQ = dq[:, None]

        s11 = np.full((NB, BL), INIT_COV)
        s12 = np.zeros((NB, BL))
        s22 = np.full((NB, BL), INIT_COV)
        m1 = np.zeros((NB, BL))
        m2 = np.zeros((NB, BL))
        e1s = eps[cidx * BL:(cidx + 1) * BL, :, :, 0].transpose(2, 0, 1)  # (NB,BL,T)? no:
        # eps slice shape (BL, T, NB, 2) -> transpose to (NB, BL, T)
        ecore = eps[cidx * BL:(cidx + 1) * BL]
        e1s = ecore[..., 0].transpose(2, 0, 1)
        e2s = ecore[..., 1].transpose(2, 0, 1)

        z1 = np.empty((NB, BL, T))
        z2 = np.empty((NB, BL, T))
        for t in range(T):
            G1, G2 = g1[:, :, t], g2[:, :, t]
            a1 = s11 * G1
            a2 = s22 * G2
            s12sq = s12 * s12
            detM = (1 + a1) * (1 + a2) - s12sq * G1 * G2
            inv = 1.0 / detM
            detS = s11 * s22 - s12sq
            sf11 = (s11 + G2 * detS) * inv
            sf22 = (s22 + G1 * detS) * inv
            sf12 = s12 * inv
            t1 = 1 + a1
            t2 = 1 + a2
            mf1 = (t2 * m1 - s12 * G2 * m2) * inv + gf1[:, :, t]
            mf2 = (-s12 * G1 * m1 + t1 * m2) * inv + gf2[:, :, t]
            # KL contribution (prior = s11,s12,s22 / m1,m2)
            d1 = m1 - mf1
            d2 = m2 - mf2
            A1 = sf11 + d1 * d1
            A2 = sf22 + d2 * d2
            Cc = sf12 + d1 * d2
            nn = s22 * A1 + s11 * A2 - 2 * s12 * Cc
            kl_sum += np.sum(nn / detS + np.log(detM))
            # sample
            l11 = np.sqrt(sf11)
            l21 = sf12 / l11
            l22 = np.sqrt(sf22 - l21 * l21)
            z1[:, :, t] = mf1 + l11 * e1s[:, :, t]
            z2[:, :, t] = mf2 + l21 * e1s[:, :, t] + l22 * e2s[:, :, t]
            # predict
            m1n = RC * mf1 - RS * mf2
            m2n = RS * mf1 + RC * mf2
            nsum = sf11 + sf22
            ndif = sf11 - sf22
            e1x = R2 * nsum
            difx = DQ * ndif - 4 * P12c * sf12
            s11 = 0.5 * (e1x + difx) + Q
            s22 = 0.5 * (e1x - difx) + Q
            s12 = P12c * ndif + DQ * sf12
            m1, m2 = m1n, m2n

        # assemble z_T (LAT, NTOK): row 2z+c, col (b*T+t)
        zT = np.empty((LAT, NTOK), np.float32)
        zT[0::2] = z1.reshape(NB, NTOK)
        zT[1::2] = z2.reshape(NB, NTOK)
        z_all.append(zT)

    # ---- device: decoder ----
    feeds = [dict(x=z_all[cidx], w1=V1, b1=c1v, w2=V2, b2=c2v)
             for cidx in range(NCORES)]
    rec_outs = _run(dec_prog, feeds)  # (OBS, NTOK)

    # ---- host: loss reductions ----
    quad = 0.0
    ivar = np.exp(-2.0 * log_R)  # (OBS,)
    for cidx in range(NCORES):
        tgt = target_seq[cidx * BL:(cidx + 1) * BL].reshape(NTOK, OBS).T
        d = tgt.astype(np.float64) - rec_outs[cidx].astype(np.float64)
        quad += np.sum((d * d) * ivar[:, None])

    n_el = B * T * NB
    loss_kl = (0.5 * kl_sum - n_el) / B
    const = B * T * OBS * 0.5 * math.log(2 * math.pi) + B * T * np.sum(log_R)
    loss_int = (const + 0.5 * quad) / B
    total = loss_kl + loss_int
    return np.array([total, loss_kl, loss_int], np.float32)
